# revision 3
# baseline (speedup 1.0000x reference)
"""DNPUConv2d Trainium2 kernel (8 NeuronCores, batch-parallel).

Restructure of the reference computation:
  - The per-device electrode permutation is folded into W1 by row
    permutation: z1 = u_d @ A_d + controls @ C_d, with A_d/C_d the
    data/control rows of the permuted W1.
  - Control contribution cb[o,i,d,:] = all_controls[o,i,d] @ C_d + b1 is
    precomputed on host (384 tiny vectors) and folded into the L1 matmul
    as an extra "ones row" of the rhs.
  - ELU is computed with the exact identity
        elu(z) = max(min(exp(z), 1), z + 1) - 1
    The "-1" is folded into the next layer's bias, so on device each
    layer computes g = max(min(exp(z'-1), 1), z') where z' = z + 1, and
    biases are carried as an extra ones-row through the matmuls.
  - The final sum over (in_ch, device) is moved before the W3 dot
    (linearity), implemented as 12 PSUM-accumulating matmuls per (b, o).

Sharding: batch 16 -> 2 per core across 8 cores. Weights replicated.

Device pipeline per core (384 tiles of [91, 512]):
  PE:  z1' = lhs1(o,i,d)^T @ [u;1]   (K=4)   -> PSUM
  ACT: e = Exp(z1' - 1)                       PSUM -> SBUF
  DVE: g1 = (e min 1) max z1'                 fused scalar_tensor_tensor
  PE:  z2' = w2g^T @ g1               (K=91)  -> PSUM
  ACT/DVE: same ELU ops -> g2
  PE:  acc += w3g^T @ g2              (M=1, PSUM accumulation x12)
"""
import numpy as np

K = 3
PAD = 1
IN_CH = 8
OUT_CH = 16
DEV = 3
N_IN = 3
N_CTRL = 4
HID = 90
B = 16
HW = 16
L = HW * HW            # 256 output positions
N_CORES = 8
B_LOC = B // N_CORES   # 2 batches per core
M = HID + 1            # 91: hidden + ones row
NT = 12                # (i,d) combo pairs per (b,o): 24 combos / 2
X = 512                # tile free dim: two L-column blocks

_COMPILED = {}


def _build_program():
    import concourse.bacc as bacc
    import concourse.tile as tile
    from concourse import mybir

    f32 = mybir.dt.float32
    f32r = mybir.dt.float32r
    Exp = mybir.ActivationFunctionType.Exp
    amin = mybir.AluOpType.min
    amax = mybir.AluOpType.max

    RCOLS = B_LOC * IN_CH * L     # 4096 rhs columns per core

    nc = bacc.Bacc()
    rhs1_d = nc.dram_tensor("rhs1", [DEV, 4, RCOLS], f32r, kind="ExternalInput")
    lhs1_d = nc.dram_tensor("lhs1", [4, OUT_CH * IN_CH * DEV, M], f32r,
                            kind="ExternalInput")
    w2g_d = nc.dram_tensor("w2g", [M, M], f32r, kind="ExternalInput")
    w3g_d = nc.dram_tensor("w3g", [M, 1], f32r, kind="ExternalInput")
    out_d = nc.dram_tensor("out", [B_LOC * OUT_CH, X], f32, kind="ExternalOutput")

    with tile.TileContext(nc) as tc:
        with (
            tc.tile_pool(name="singles", bufs=1) as singles,
            tc.tile_pool(name="work", bufs=3) as work,
            tc.tile_pool(name="outp", bufs=2) as outp,
            tc.tile_pool(name="psz1", bufs=3, space="PSUM") as psz1,
            tc.tile_pool(name="psz2", bufs=3, space="PSUM") as psz2,
            tc.tile_pool(name="psacc", bufs=2, space="PSUM") as psacc,
        ):
            rhs_sb = [singles.tile([4, RCOLS], f32r, tag=f"rhs{d}",
                                   name=f"rhs_sb{d}")
                      for d in range(DEV)]
            w2g_sb = singles.tile([M, M], f32r)
            w3g_sb = singles.tile([M, 1], f32r)
            neg1 = singles.tile([128, 1], f32)
            nc.vector.memset(neg1, -1.0)
            for d in range(DEV):
                nc.sync.dma_start(out=rhs_sb[d], in_=rhs1_d[d])
            nc.sync.dma_start(out=w2g_sb, in_=w2g_d[:, :])
            nc.sync.dma_start(out=w3g_sb, in_=w3g_d[:, :])

            NJ = IN_CH * DEV      # 24 (i,d) combos per o
            for o in range(OUT_CH):
                lhs_o = outp.tile([4, NJ, M], f32r, tag="lhs_o", name="lhs_o")
                nc.sync.dma_start(out=lhs_o,
                                  in_=lhs1_d[:, o * NJ:(o + 1) * NJ, :])
                for b in range(B_LOC):
                    acc = psacc.tile([1, X], f32)
                    for t in range(NT):
                        z1 = psz1.tile([M, X], f32)
                        for h in range(2):
                            j = 2 * t + h
                            i, d = j // DEV, j % DEV
                            col = (b * IN_CH + i) * L
                            nc.tensor.matmul(
                                z1[:, h * L:(h + 1) * L],
                                lhs_o[:, j, :],
                                rhs_sb[d][:, col:col + L],
                                start=True, stop=True,
                            )
                        e1 = work.tile([M, X], f32, tag="e1")
                        nc.scalar.activation(e1, z1, Exp, bias=neg1[:M], scale=1.0)
                        g1 = work.tile([M, X], f32r, tag="g1")
                        nc.vector.scalar_tensor_tensor(
                            out=g1, in0=e1, scalar=1.0, in1=z1,
                            op0=amin, op1=amax)
                        z2 = psz2.tile([M, X], f32)
                        nc.tensor.matmul(z2, w2g_sb, g1, start=True, stop=True)
                        e2 = work.tile([M, X], f32, tag="e2")
                        nc.scalar.activation(e2, z2, Exp, bias=neg1[:M], scale=1.0)
                        g2 = work.tile([M, X], f32r, tag="g2")
                        nc.vector.scalar_tensor_tensor(
                            out=g2, in0=e2, scalar=1.0, in1=z2,
                            op0=amin, op1=amax)
                        nc.tensor.matmul(acc, w3g_sb, g2,
                                         start=(t == 0), stop=(t == NT - 1))
                    bo = b * OUT_CH + o
                    out_sb = outp.tile([1, X], f32, tag="osb")
                    nc.scalar.copy(out_sb, acc)
                    nc.sync.dma_start(out=out_d[bo:bo + 1, :], in_=out_sb)

    nc.compile()
    return nc


def _get_program():
    if "nc" not in _COMPILED:
        _COMPILED["nc"] = _build_program()
    return _COMPILED["nc"]


def _host_prep(x, all_controls, W1, b1, W2, b2, W3, b3,
               input_indices, control_indices):
    """Build per-core input maps from the full problem inputs."""
    x = np.asarray(x, np.float32)
    ac = np.asarray(all_controls, np.float32)
    W1 = np.asarray(W1, np.float32); b1 = np.asarray(b1, np.float32)
    W2 = np.asarray(W2, np.float32); b2 = np.asarray(b2, np.float32)
    W3 = np.asarray(W3, np.float32); b3 = np.asarray(b3, np.float32)
    ii = np.asarray(input_indices).astype(np.int64)
    ci = np.asarray(control_indices).astype(np.int64)

    # unfold (torch F.unfold ordering), pad=1, k=3, stride=1
    xp = np.pad(x, ((0, 0), (0, 0), (PAD, PAD), (PAD, PAD)))
    cols = [xp[:, :, i:i + HW, j:j + HW] for i in range(K) for j in range(K)]
    u = np.stack(cols, axis=2).reshape(B, IN_CH, K * K, L)
    u = u.transpose(0, 1, 3, 2).reshape(B, IN_CH, L, DEV, N_IN)

    # permuted W1 rows
    idx = np.concatenate([ii, ci], axis=-1)           # [DEV, 7]
    Wp = np.zeros((DEV, N_IN + N_CTRL, HID), np.float32)
    for d in range(DEV):
        for e in range(N_IN + N_CTRL):
            Wp[d, idx[d, e], :] = W1[e, :]
    A = Wp[:, :N_IN, :]                               # [DEV, 3, 90]
    C = Wp[:, N_IN:, :]                               # [DEV, 4, 90]
    cb = np.einsum('oidc,dch->oidh', ac, C) + b1      # [O, I, DEV, 90]
    b2f = b2 - W2.sum(axis=0)
    b3f = float((b3 - W3.sum(axis=0))[0])

    # lhs1: [4, O*I*DEV, 91]
    lhs1 = np.zeros((4, OUT_CH * IN_CH * DEV, M), np.float32)
    for o in range(OUT_CH):
        for i in range(IN_CH):
            for d in range(DEV):
                oid = (o * IN_CH + i) * DEV + d
                lhs1[:N_IN, oid, :HID] = A[d]
                lhs1[N_IN, oid, :HID] = cb[o, i, d] + 1.0
                lhs1[N_IN, oid, HID] = 1.0
    w2g = np.zeros((M, M), np.float32)
    w2g[:HID, :HID] = W2
    w2g[HID, :HID] = b2f + 1.0
    w2g[HID, HID] = 1.0
    w3g = np.zeros((M, 1), np.float32)
    w3g[:HID, 0] = W3[:, 0]
    w3g[HID, 0] = b3f

    in_maps = []
    for c in range(N_CORES):
        ub = u[c * B_LOC:(c + 1) * B_LOC]             # [2, I, L, DEV, 3]
        rhs1 = np.empty((DEV, 4, B_LOC * IN_CH * L), np.float32)
        for d in range(DEV):
            # rows 0..2: u values, row 3: ones; cols = (b, i, l)
            rhs1[d, :N_IN] = ub[:, :, :, d, :].transpose(3, 0, 1, 2).reshape(
                N_IN, B_LOC * IN_CH * L)
            rhs1[d, N_IN] = 1.0
        in_maps.append({"rhs1": rhs1, "lhs1": lhs1, "w2g": w2g, "w3g": w3g})
    return in_maps


def kernel(x, all_controls, W1, b1, W2, b2, W3, b3,
           input_indices, control_indices):
    from concourse.bass_utils import run_bass_kernel_spmd

    nc = _get_program()
    in_maps = _host_prep(x, all_controls, W1, b1, W2, b2, W3, b3,
                         input_indices, control_indices)
    res = run_bass_kernel_spmd(nc, in_maps, list(range(N_CORES)))
    out = np.empty((B, OUT_CH, HW, HW), np.float32)
    for c in range(N_CORES):
        o_c = res.results[c]["out"].reshape(B_LOC, OUT_CH, 2, L)
        out[c * B_LOC:(c + 1) * B_LOC] = (
            o_c[:, :, 0, :] + o_c[:, :, 1, :]).reshape(B_LOC, OUT_CH, HW, HW)
    return out
